# revision 2
# baseline (speedup 1.0000x reference)
"""Trainium2 Bass kernel for nn_CategoryAdder (embedding lookup + masked add).

Computation: out[b,s,:] = inputs[b,s,:] + emb where
  emb = table[categories[b,s]] masked to zero when categories[b,s]==0 or
  s == mask_positions[b].

Host-side preprocessing folds both masks into the data:
  - categories[b, mask_positions[b]] = 0
  - table row 0 zeroed (on a copy)
so the device computes exactly: out = inputs + table0[categories].

Design (v2): the baseline SWDGE dma_gather moved the table rows over the DMA
bus (fp32: 32 MB/core of random 2 KB reads), leaving the kernel DMA-bus-bound
at ~100 MB/core (346 us measured ~= the 360 GB/s bus floor). This version
cuts DMA bytes ~2.6x:
  - everything is bf16 on device (rel tolerance 2e-2 >> bf16's ~3e-3);
  - the (5000,512) table lives in SBUF feature-major ([128 part, 5000 cat,
    4 feat] = 40 KB/partition) and rows are gathered by the GPSIMD ap_gather
    ucode instead of DMA, so the gather never touches the DMA bus;
  - x/out are pre/post-transposed on the host to the same feature-major
    layout so device DMA is purely contiguous bf16 streams.
Per-core DMA: 16.8 (x) + 16.8 (out) + 5.1 (table) = 38.8 MB -> ~108 us floor;
GPSIMD gather ~5.6 ns/token (cost model) -> ~91 us, overlapped.

Sharding: data-parallel over batch across 8 NeuronCores (8 batches per core,
16384 tokens/core). Table replicated.
"""

import numpy as np
import ml_dtypes

import concourse.mybir as mybir
from concourse import bacc, tile
from concourse.bass_utils import run_bass_kernel_spmd

BF16 = ml_dtypes.bfloat16


def _ensure_axon_ntff_hook_module():
    """run_bass_kernel_spmd(trace=True) under axon imports antenv.axon_hooks,
    which this image lacks — install a fallback shim (backed by the boot
    module's ctypes hook when available) so a BASS_TRACE=1 environment does
    not crash the kernel. No-op when the real module exists."""
    try:
        import antenv.axon_hooks  # noqa: F401
        return
    except ImportError:
        pass
    import sys
    import types

    hook = None
    try:
        import trn_agent_boot.trn_boot as _tb

        hook = _tb._ntff_profile_via_ctypes("/opt/axon/libaxon_pjrt.so")
    except Exception:
        hook = None  # get_..._hook() -> None makes bass_utils skip tracing
    mod = types.ModuleType("antenv.axon_hooks")
    mod.get_axon_ntff_profile_hook = lambda: hook
    mod.set_axon_ntff_profile_hook = lambda h: None
    sys.modules["antenv.axon_hooks"] = mod


_ensure_axon_ntff_hook_module()

B, S, D = 64, 2048, 512
N_CAT = 5000
N_CORES = 8
B_PER = B // N_CORES          # 8 batches per core
NTOK = B_PER * S              # 16384 tokens per core
DP = D // 128                 # features per partition (4)
T = 1024                      # tokens per tile
NTILES = NTOK // T
IDX_COLS = NTOK // 16         # wrapped int16 index columns


def _build_nc():
    nc = bacc.Bacc("TRN2", target_bir_lowering=False, debug=False)
    xt = nc.dram_tensor("xt", [128, NTOK * DP], mybir.dt.bfloat16,
                        kind="ExternalInput")
    tblt = nc.dram_tensor("tblt", [128, N_CAT * DP], mybir.dt.bfloat16,
                          kind="ExternalInput")
    idx = nc.dram_tensor("idx", [128, IDX_COLS], mybir.dt.int16,
                         kind="ExternalInput")
    out = nc.dram_tensor("out", [128, NTOK * DP], mybir.dt.bfloat16,
                         kind="ExternalOutput")

    with tile.TileContext(nc) as tc:
        with (
            tc.tile_pool(name="tblp", bufs=1) as tblp,
            tc.tile_pool(name="idxp", bufs=1) as idxp,
            tc.tile_pool(name="inp", bufs=4) as inp,
            tc.tile_pool(name="embp", bufs=4) as embp,
        ):
            idx_sb = idxp.tile([128, IDX_COLS], mybir.dt.int16)
            nc.sync.dma_start(out=idx_sb[:], in_=idx[:, :])
            tbl_sb = tblp.tile([128, N_CAT * DP], mybir.dt.bfloat16)
            nc.sync.dma_start(out=tbl_sb[:], in_=tblt[:, :])
            for t in range(NTILES):
                c0 = t * T * DP
                emb_t = embp.tile([128, T * DP], mybir.dt.bfloat16, tag="emb")
                nc.gpsimd.ap_gather(
                    emb_t[:],
                    tbl_sb[:],
                    idx_sb[:, t * (T // 16): (t + 1) * (T // 16)],
                    channels=128,
                    num_elems=N_CAT,
                    d=DP,
                    num_idxs=T,
                )
                x_t = inp.tile([128, T * DP], mybir.dt.bfloat16, tag="in")
                nc.sync.dma_start(out=x_t[:], in_=xt[:, c0: c0 + T * DP])
                nc.vector.tensor_add(out=x_t[:], in0=x_t[:], in1=emb_t[:])
                nc.sync.dma_start(out=out[:, c0: c0 + T * DP], in_=x_t[:])
    nc.compile()
    return nc


def _prep_idx(cat_shard: np.ndarray) -> np.ndarray:
    """cat_shard: (NTOK,) -> wrapped int16 [128, IDX_COLS] for ap_gather.

    Per tile of T tokens the gather consumes idxs[p % 16, s] for token
    s*16 + (p % 16); the same block is replicated across the 8 groups of 16
    partitions (each Q7 core reads its own group).
    """
    w = cat_shard.reshape(NTILES, T // 16, 16).transpose(1, 0, 2)
    # w[s, tile, p] = cat[tile*T + s*16 + p] -> [16, NTILES * T//16]
    w = w.transpose(2, 1, 0).reshape(16, IDX_COLS)
    return np.ascontiguousarray(np.tile(w, (8, 1)).astype(np.int16))


RUN_KWARGS = {}  # test harness can set e.g. {"trace": True}
LAST_RESULTS = None
_NC = None


def _get_nc():
    global _NC
    if _NC is None:
        _NC = _build_nc()
    return _NC


def kernel(inputs, categories, mask_positions, table):
    global LAST_RESULTS
    inputs = np.asarray(inputs, dtype=np.float32)
    categories = np.asarray(categories).astype(np.int64)
    mask_positions = np.asarray(mask_positions).astype(np.int64)
    table = np.asarray(table, dtype=np.float32)

    # Fold both masks into the data.
    cat = categories.copy()
    cat[np.arange(B), mask_positions[:, 0]] = 0
    tbl0 = table.astype(BF16)
    tbl0[0] = 0.0
    # feature-major: [128, N_CAT, DP] -> [128, N_CAT*DP]
    tblt = np.ascontiguousarray(
        tbl0.reshape(N_CAT, 128, DP).transpose(1, 0, 2)
    ).reshape(128, N_CAT * DP)

    x16 = inputs.astype(BF16)  # one fp32->bf16 pass over the full input

    nc = _get_nc()

    in_maps = []
    for c in range(N_CORES):
        xc = x16[c * B_PER: (c + 1) * B_PER].reshape(NTOK, 128, DP)
        xt = np.ascontiguousarray(xc.transpose(1, 0, 2)).reshape(128, NTOK * DP)
        cat_shard = cat[c * B_PER: (c + 1) * B_PER].reshape(NTOK)
        in_maps.append({"xt": xt, "tblt": tblt, "idx": _prep_idx(cat_shard)})

    res = run_bass_kernel_spmd(
        nc, in_maps, core_ids=list(range(N_CORES)), **RUN_KWARGS
    )
    LAST_RESULTS = res
    outs = []
    for r in res.results:
        o = np.asarray(r["out"]).reshape(128, NTOK, DP).transpose(1, 0, 2)
        outs.append(o.reshape(NTOK, D).astype(np.float32).reshape(B_PER, S, D))
    return np.concatenate(outs, axis=0)


# revision 3
# speedup vs baseline: 3.2598x; 3.2598x over previous
"""Trainium2 Bass kernel for nn_CategoryAdder (embedding lookup + masked add).

Computation: out[b,s,:] = inputs[b,s,:] + emb where
  emb = table[categories[b,s]] masked to zero when categories[b,s]==0 or
  s == mask_positions[b].

Host-side preprocessing folds both masks into the data:
  - categories[b, mask_positions[b]] = 0
  - table row 0 zeroed (on a copy)
so the device computes exactly: out = inputs + table0[categories].

v3: the fp32 baseline was DMA-bus-bound (100.9 MB/core over the 16-engine
~360 GB/s bus -> 346 us). Everything on device is bf16 now (rel tolerance
2e-2 >> bf16's ~2.5e-3), halving bus bytes to 50.4 MB/core (~140 us floor):
x and out stream as bf16, and the SWDGE dma_gather pulls 1 KB bf16 table
rows. Q7 descriptor generation (~8.6 ns/idx, 141 us serial) is spread
round-robin across 4 SWDGE queues — each queue's desc-gen runs on its own
pair of the 8 Q7 cores — to pull it off the critical path.

Sharding: data-parallel over batch across 8 NeuronCores (8 batches per core,
16384 tokens/core). Table replicated.
"""

import numpy as np
import ml_dtypes

import concourse.mybir as mybir
from concourse import bacc, tile
from concourse.bass_utils import run_bass_kernel_spmd

BF16 = ml_dtypes.bfloat16


def _ensure_axon_ntff_hook_module():
    """run_bass_kernel_spmd(trace=True) under axon imports antenv.axon_hooks,
    which this image lacks — install a fallback shim (backed by the boot
    module's ctypes hook when available) so a BASS_TRACE=1 environment does
    not crash the kernel. No-op when the real module exists."""
    try:
        import antenv.axon_hooks  # noqa: F401
        return
    except ImportError:
        pass
    import sys
    import types

    hook = None
    try:
        import trn_agent_boot.trn_boot as _tb

        hook = _tb._ntff_profile_via_ctypes("/opt/axon/libaxon_pjrt.so")
    except Exception:
        hook = None  # get_..._hook() -> None makes bass_utils skip tracing
    mod = types.ModuleType("antenv.axon_hooks")
    mod.get_axon_ntff_profile_hook = lambda: hook
    mod.set_axon_ntff_profile_hook = lambda h: None
    sys.modules["antenv.axon_hooks"] = mod


_ensure_axon_ntff_hook_module()

B, S, D = 64, 2048, 512
N_CAT = 5000
N_CORES = 8
B_PER = B // N_CORES          # 8 batches per core
NTOK = B_PER * S              # 16384 tokens per core
IDX_COLS = NTOK // 16         # columns of the wrapped int16 index tensor
N_QUEUES = 4

# Tile schedule (tokens per tile): small tiles prime the pipeline at the start
# and shorten the serial add+store chain at the end.
TILES = [256, 256, 512] + [1024] * 14 + [512, 512]
assert sum(TILES) == NTOK


def _build_nc():
    nc = bacc.Bacc(
        "TRN2", target_bir_lowering=False, debug=False, num_swdge_queues=N_QUEUES
    )
    x = nc.dram_tensor("x", [NTOK, D], mybir.dt.bfloat16, kind="ExternalInput")
    tbl = nc.dram_tensor("tbl", [N_CAT, D], mybir.dt.bfloat16, kind="ExternalInput")
    idx = nc.dram_tensor("idx", [128, IDX_COLS], mybir.dt.int16, kind="ExternalInput")
    out = nc.dram_tensor("out", [NTOK, D], mybir.dt.bfloat16, kind="ExternalOutput")

    with tile.TileContext(nc) as tc:
        with (
            tc.tile_pool(name="idxp", bufs=1) as idxp,
            tc.tile_pool(name="inp", bufs=6) as inp,
            tc.tile_pool(name="embp", bufs=6) as embp,
        ):
            idx_sb = idxp.tile([128, IDX_COLS], mybir.dt.int16)
            # Load the head tiles' indices as a small separate DMA so the
            # first gathers are not gated on the full idx transfer.
            head = sum(t // 16 for t in TILES[:4])
            nc.sync.dma_start(out=idx_sb[:, :head], in_=idx[:, :head])
            nc.sync.dma_start(out=idx_sb[:, head:], in_=idx[:, head:])
            t0 = 0
            col = 0
            for i, T in enumerate(TILES):
                C = T // 128
                emb_t = embp.tile([128, C * D], mybir.dt.bfloat16, tag="emb")
                nc.gpsimd.dma_gather(
                    emb_t[:].rearrange("p (c e) -> p c e", e=D),
                    tbl[:, :],
                    idx_sb[:, col : col + T // 16],
                    T,
                    T,
                    D,
                    single_packet=False,
                    queue_num=i % N_QUEUES,
                )
                in_t = inp.tile([128, C * D], mybir.dt.bfloat16, tag="in")
                nc.sync.dma_start(
                    out=in_t[:],
                    in_=x[t0 : t0 + T].rearrange("(p c) e -> p (c e)", p=128),
                )
                nc.vector.tensor_add(out=in_t[:], in0=in_t[:], in1=emb_t[:])
                nc.sync.dma_start(
                    out=out[t0 : t0 + T].rearrange("(p c) e -> p (c e)", p=128),
                    in_=in_t[:],
                )
                t0 += T
                col += T // 16
    nc.compile()
    return nc


def _prep_idx(cat_shard: np.ndarray) -> np.ndarray:
    """cat_shard: (NTOK,) int -> wrapped int16 index tensor [128, IDX_COLS].

    dma_gather writes gather-slot i to SBUF (partition i%128, column i//128);
    our tiles place token t at (partition t//C, column t%C), so slot i holds
    the category of token (i%128)*C + i//128. Indices are then wrapped 16-way
    (idxs[p, s] = slot s*16+p) and replicated across the 8 groups of 16
    partitions so any SWDGE queue's core pair reads the same list.
    """
    blocks = []
    t0 = 0
    for T in TILES:
        C = T // 128
        slot_to_token = (np.arange(T) % 128) * C + (np.arange(T) // 128)
        vals = cat_shard[t0 : t0 + T][slot_to_token]
        blocks.append(np.tile(vals.reshape(T // 16, 16).T, (8, 1)))
        t0 += T
    return np.ascontiguousarray(np.concatenate(blocks, axis=1).astype(np.int16))


RUN_KWARGS = {}  # test harness can set e.g. {"trace": True}
LAST_RESULTS = None
_NC = None


def _get_nc():
    global _NC
    if _NC is None:
        _NC = _build_nc()
    return _NC


def kernel(inputs, categories, mask_positions, table):
    global LAST_RESULTS
    inputs = np.asarray(inputs, dtype=np.float32)
    categories = np.asarray(categories).astype(np.int64)
    mask_positions = np.asarray(mask_positions).astype(np.int64)
    table = np.asarray(table, dtype=np.float32)

    # Fold both masks into the data.
    cat = categories.copy()
    cat[np.arange(B), mask_positions[:, 0]] = 0
    tbl0 = table.astype(BF16)
    tbl0[0] = 0.0

    x16 = inputs.astype(BF16)  # one fp32->bf16 pass over the full input

    nc = _get_nc()

    in_maps = []
    for c in range(N_CORES):
        x_shard = np.ascontiguousarray(
            x16[c * B_PER : (c + 1) * B_PER].reshape(NTOK, D)
        )
        cat_shard = cat[c * B_PER : (c + 1) * B_PER].reshape(NTOK)
        in_maps.append({"x": x_shard, "tbl": tbl0, "idx": _prep_idx(cat_shard)})

    res = run_bass_kernel_spmd(
        nc, in_maps, core_ids=list(range(N_CORES)), **RUN_KWARGS
    )
    LAST_RESULTS = res
    out = np.concatenate(
        [
            np.asarray(r["out"]).astype(np.float32).reshape(B_PER, S, D)
            for r in res.results
        ],
        axis=0,
    )
    return out


# revision 4
# speedup vs baseline: 3.2741x; 1.0044x over previous
"""Trainium2 Bass kernel for nn_CategoryAdder (embedding lookup + masked add).

Computation: out[b,s,:] = inputs[b,s,:] + emb where
  emb = table[categories[b,s]] masked to zero when categories[b,s]==0 or
  s == mask_positions[b].

Host-side preprocessing folds both masks into the data:
  - categories[b, mask_positions[b]] = 0
  - table row 0 zeroed (on a copy)
so the device computes exactly: out = inputs + table0[categories].

v3: the fp32 baseline was DMA-bus-bound (100.9 MB/core over the 16-engine
~360 GB/s bus -> 346 us). Everything on device is bf16 now (rel tolerance
2e-2 >> bf16's ~2.5e-3), halving bus bytes to 50.4 MB/core (~140 us floor):
x and out stream as bf16, and the SWDGE dma_gather pulls 1 KB bf16 table
rows. Q7 descriptor generation (~8.6 ns/idx, 141 us serial) is spread
round-robin across 4 SWDGE queues — each queue's desc-gen runs on its own
pair of the 8 Q7 cores — to pull it off the critical path.

Sharding: data-parallel over batch across 8 NeuronCores (8 batches per core,
16384 tokens/core). Table replicated.
"""

import numpy as np
import ml_dtypes

import concourse.mybir as mybir
from concourse import bacc, tile
from concourse.bass_utils import run_bass_kernel_spmd

BF16 = ml_dtypes.bfloat16


def _ensure_axon_ntff_hook_module():
    """run_bass_kernel_spmd(trace=True) under axon imports antenv.axon_hooks,
    which this image lacks — install a fallback shim (backed by the boot
    module's ctypes hook when available) so a BASS_TRACE=1 environment does
    not crash the kernel. No-op when the real module exists."""
    try:
        import antenv.axon_hooks  # noqa: F401
        return
    except ImportError:
        pass
    import sys
    import types

    hook = None
    try:
        import trn_agent_boot.trn_boot as _tb

        hook = _tb._ntff_profile_via_ctypes("/opt/axon/libaxon_pjrt.so")
    except Exception:
        hook = None  # get_..._hook() -> None makes bass_utils skip tracing
    mod = types.ModuleType("antenv.axon_hooks")
    mod.get_axon_ntff_profile_hook = lambda: hook
    mod.set_axon_ntff_profile_hook = lambda h: None
    sys.modules["antenv.axon_hooks"] = mod


_ensure_axon_ntff_hook_module()

B, S, D = 64, 2048, 512
N_CAT = 5000
N_CORES = 8
B_PER = B // N_CORES          # 8 batches per core
NTOK = B_PER * S              # 16384 tokens per core
IDX_COLS = NTOK // 16         # columns of the wrapped int16 index tensor
N_QUEUES = 4

# Uniform tiles; all gathers are issued up-front (embp bufs covers every
# tile) so 4-queue desc-gen runs flat out and gather SDMA traffic is fully
# decoupled from the load/add/store pipeline.
TILES = [1024] * 16
assert sum(TILES) == NTOK


def _build_nc():
    nc = bacc.Bacc(
        "TRN2", target_bir_lowering=False, debug=False, num_swdge_queues=N_QUEUES
    )
    x = nc.dram_tensor("x", [NTOK, D], mybir.dt.bfloat16, kind="ExternalInput")
    tbl = nc.dram_tensor("tbl", [N_CAT, D], mybir.dt.bfloat16, kind="ExternalInput")
    idx = nc.dram_tensor("idx", [128, IDX_COLS], mybir.dt.int16, kind="ExternalInput")
    out = nc.dram_tensor("out", [NTOK, D], mybir.dt.bfloat16, kind="ExternalOutput")

    with tile.TileContext(nc) as tc:
        with (
            tc.tile_pool(name="idxp", bufs=1) as idxp,
            tc.tile_pool(name="inp", bufs=8) as inp,
            tc.tile_pool(name="embp", bufs=len(TILES)) as embp,
        ):
            idx_sb = idxp.tile([128, IDX_COLS], mybir.dt.int16)
            nc.sync.dma_start(out=idx_sb[:], in_=idx[:, :])
            emb_tiles = []
            col = 0
            for i, T in enumerate(TILES):
                C = T // 128
                emb_t = embp.tile([128, C * D], mybir.dt.bfloat16, tag="emb")
                nc.gpsimd.dma_gather(
                    emb_t[:].rearrange("p (c e) -> p c e", e=D),
                    tbl[:, :],
                    idx_sb[:, col : col + T // 16],
                    T,
                    T,
                    D,
                    single_packet=False,
                    queue_num=i % N_QUEUES,
                )
                emb_tiles.append(emb_t)
                col += T // 16
            t0 = 0
            for i, T in enumerate(TILES):
                C = T // 128
                in_t = inp.tile([128, C * D], mybir.dt.bfloat16, tag="in")
                nc.sync.dma_start(
                    out=in_t[:],
                    in_=x[t0 : t0 + T].rearrange("(p c) e -> p (c e)", p=128),
                )
                nc.vector.tensor_add(out=in_t[:], in0=in_t[:], in1=emb_tiles[i][:])
                # stores ride the scalar engine's HWDGE queue so the late
                # store stream never head-of-line blocks the early loads
                nc.scalar.dma_start(
                    out=out[t0 : t0 + T].rearrange("(p c) e -> p (c e)", p=128),
                    in_=in_t[:],
                )
                t0 += T
    nc.compile()
    return nc


def _prep_idx(cat_shard: np.ndarray) -> np.ndarray:
    """cat_shard: (NTOK,) int -> wrapped int16 index tensor [128, IDX_COLS].

    dma_gather writes gather-slot i to SBUF (partition i%128, column i//128);
    our tiles place token t at (partition t//C, column t%C), so slot i holds
    the category of token (i%128)*C + i//128. Indices are then wrapped 16-way
    (idxs[p, s] = slot s*16+p) and replicated across the 8 groups of 16
    partitions so any SWDGE queue's core pair reads the same list.
    """
    blocks = []
    t0 = 0
    for T in TILES:
        C = T // 128
        slot_to_token = (np.arange(T) % 128) * C + (np.arange(T) // 128)
        vals = cat_shard[t0 : t0 + T][slot_to_token]
        blocks.append(np.tile(vals.reshape(T // 16, 16).T, (8, 1)))
        t0 += T
    return np.ascontiguousarray(np.concatenate(blocks, axis=1).astype(np.int16))


RUN_KWARGS = {}  # test harness can set e.g. {"trace": True}
LAST_RESULTS = None
_NC = None


def _get_nc():
    global _NC
    if _NC is None:
        _NC = _build_nc()
    return _NC


def kernel(inputs, categories, mask_positions, table):
    global LAST_RESULTS
    inputs = np.asarray(inputs, dtype=np.float32)
    categories = np.asarray(categories).astype(np.int64)
    mask_positions = np.asarray(mask_positions).astype(np.int64)
    table = np.asarray(table, dtype=np.float32)

    # Fold both masks into the data.
    cat = categories.copy()
    cat[np.arange(B), mask_positions[:, 0]] = 0
    tbl0 = table.astype(BF16)
    tbl0[0] = 0.0

    x16 = inputs.astype(BF16)  # one fp32->bf16 pass over the full input

    nc = _get_nc()

    in_maps = []
    for c in range(N_CORES):
        x_shard = np.ascontiguousarray(
            x16[c * B_PER : (c + 1) * B_PER].reshape(NTOK, D)
        )
        cat_shard = cat[c * B_PER : (c + 1) * B_PER].reshape(NTOK)
        in_maps.append({"x": x_shard, "tbl": tbl0, "idx": _prep_idx(cat_shard)})

    res = run_bass_kernel_spmd(
        nc, in_maps, core_ids=list(range(N_CORES)), **RUN_KWARGS
    )
    LAST_RESULTS = res
    out = np.concatenate(
        [
            np.asarray(r["out"]).astype(np.float32).reshape(B_PER, S, D)
            for r in res.results
        ],
        axis=0,
    )
    return out
